# revision 50
# baseline (speedup 1.0000x reference)
"""BiasAdd + LayerNorm + FP8 quantization kernel for Trainium2 (Bass/Tile).

Reference computation (see problem reference.py):
    bda  = residual + (x + bias)                 # [B,S,H] -> flattened [B*S, H]
    ln   = layernorm(bda) * ln_weight + ln_bias  # fp32
    amax = max(|ln|)
    fp8  = clip(ln, +-448).astype(float8_e4m3fn)
    returns (bda2, fp8, amax)

Sharding: data-parallel over the flattened token dim (32768 rows) across
8 NeuronCores -> 4096 rows/core. bias/ln_weight/ln_bias replicated.
amax: per-partition partial maxima [128] per core, final max on host.

Engine split (per 2-block supertile, chosen from measured costs):
  - x+residual: computed during the residual load (SWDGE accum DMA,
    4 software-DGE queues so the ~13 us RMW transfers overlap)
  - +bias, *gamma, +beta, |ln| max, mean/var (bn_stats): DVE, batched
    across the supertile (fp32 tensor_tensor is capped at 1 elem/cycle;
    per-column operands can only run on DVE -- Pool shares DVE's SBUF
    ports and the PE's fp32 passthrough is slower, both measured)
  - sqrt, (bda-mu)*rsigma scale-shift, fp8 cast: ACT
Four-stage software pipeline across supertiles hides the cross-engine
latency. Measured ~237 us/core vs the ~150 us HBM roofline (52 MiB
per core at ~358 GB/s per-NC).
"""

import sys

import numpy as np

_TRN_REPO = "/opt/trn_rl_repo"
if _TRN_REPO not in sys.path:
    sys.path.insert(0, _TRN_REPO)

import ml_dtypes  # noqa: E402
import concourse.bass as bass  # noqa: E402
import concourse.bacc as bacc  # noqa: E402
import concourse.tile as tile  # noqa: E402
from concourse import mybir  # noqa: E402

EPS = 1e-5
H = 1024
P = 128
N_CORES = 8
R_FULL = 8 * 4096  # B * S
R_CORE = R_FULL // N_CORES  # 4096 rows per core
BLOCKS_PER_TILE = 2  # A: 128-row blocks per supertile


def _bcast(ap, dims):
    """Prepend broadcast (step-0) dims to an AP."""
    return bass.AP(
        tensor=ap.tensor,
        offset=ap.offset,
        ap=[[0, n] for n in dims] + list(ap.ap),
    )


def build_nc(rows: int = R_CORE, blocks_per_tile: int = BLOCKS_PER_TILE) -> bass.Bass:
    """One NeuronCore's program: bias-add + layernorm + fp8 over [rows, H]."""
    dt = mybir.dt
    A = blocks_per_tile
    assert rows % (P * A) == 0
    nblk = rows // P
    J = nblk // A

    # Bacc (not plain Bass): its finalize() runs generate_event_semaphores(),
    # which splits multi-semaphore waits to satisfy the 1-wait-per-instruction
    # hardware constraint that Tile-emitted code otherwise violates.
    # 4 SWDGE queues: the residual accum-DMAs would otherwise serialize on
    # one software-DGE queue (~15 us per 1 MiB RMW transfer) and pace the
    # whole pipeline.
    nc = bacc.Bacc(num_swdge_queues=4)
    x = nc.declare_dram_parameter("x", [rows, H], dt.float32, isOutput=False)
    res = nc.declare_dram_parameter("residual", [rows, H], dt.float32, isOutput=False)
    bias = nc.declare_dram_parameter("bias", [H], dt.float32, isOutput=False)
    gamma = nc.declare_dram_parameter("ln_weight", [H], dt.float32, isOutput=False)
    beta = nc.declare_dram_parameter("ln_bias", [H], dt.float32, isOutput=False)
    bda_out = nc.declare_dram_parameter("bda_out", [rows, H], dt.float32, isOutput=True)
    fp8_out = nc.declare_dram_parameter("fp8_out", [rows, H], dt.float8e4, isOutput=True)
    amax_out = nc.declare_dram_parameter("amax_out", [P, 1], dt.float32, isOutput=True)

    # [rows, H] viewed as J supertiles of [P partitions, A row-blocks, H]
    xv = x[:].rearrange("(j a p) h -> j p a h", p=P, a=A)
    rv = res[:].rearrange("(j a p) h -> j p a h", p=P, a=A)
    bv = bda_out[:].rearrange("(j a p) h -> j p a h", p=P, a=A)
    fv = fp8_out[:].rearrange("(j a p) h -> j p a h", p=P, a=A)

    with tile.TileContext(nc) as tc:
        with (
            tc.tile_pool(name="consts", bufs=1) as consts,
            tc.tile_pool(name="io", bufs=6) as io,
            tc.tile_pool(name="work", bufs=5) as work,
            tc.tile_pool(name="stats", bufs=8) as stats,
            tc.tile_pool(name="rpool", bufs=3) as rpool,
        ):
            # Broadcast bias/gamma/beta across all 128 partitions once.
            bias_b = consts.tile([P, H], dt.float32, tag="bias_b")
            gamma_b = consts.tile([P, H], dt.float32, tag="gamma_b")
            beta_b = consts.tile([P, H], dt.float32, tag="beta_b")
            for tgt, src in ((bias_b, bias), (gamma_b, gamma), (beta_b, beta)):
                nc.sync.dma_start(out=tgt, in_=_bcast(src[:], [P]))
            eps_t = consts.tile([P, 1], dt.float32, tag="eps")
            nc.vector.memset(eps_t, EPS)
            # Per-partition running |ln| maxima, one column per row-block.
            amax_acc = consts.tile([P, nblk], dt.float32, tag="amax_acc")

            gamma_ba = bass.AP(
                tensor=gamma_b.tensor,
                offset=gamma_b.offset,
                ap=[list(gamma_b.ap[0]), [0, A], list(gamma_b.ap[1])],
            )
            beta_ba = bass.AP(
                tensor=beta_b.tensor,
                offset=beta_b.offset,
                ap=[list(beta_b.ap[0]), [0, A], list(beta_b.ap[1])],
            )
            # Four-stage software pipeline over supertiles so DVE never
            # stalls on a DMA or ACT round trip:
            #   stage A (j):   x load + residual SWDGE-accum issue (prefetch)
            #   stage B (j-1): +bias (DVE), bn_stats (DVE), sqrt (ACT)
            #   stage C (j-2): recip + nmr (DVE), scale-shift (ACT)
            #   stage D (j-3): gamma/beta/|max| (DVE), cast (ACT), stores
            def stageA(j):
                xt = io.tile([P, A, H], dt.float32, tag="xt")
                nc.sync.dma_start(out=xt, in_=xv[j])
                if j < 3:
                    # ramp peeling: during pipeline fill the SWDGE accum's
                    # ~16 us latency gates the first supertiles while DVE sits
                    # idle — load residual over HWDGE (~3 us) and add on DVE
                    rt = rpool.tile([P, A, H], dt.float32, tag="rt")
                    nc.sync.dma_start(out=rt, in_=rv[j])
                    return (j, xt, rt)
                # x+residual computed by a SWDGE accum DMA
                nc.gpsimd.dma_start(out=xt, in_=rv[j], accum_op=mybir.AluOpType.add)
                return (j, xt, None)

            def stageB(s):
                j, xt, rt = s
                if rt is not None:
                    nc.vector.tensor_add(out=xt, in0=xt, in1=rt)
                bias_ba = bass.AP(
                    tensor=bias_b.tensor,
                    offset=bias_b.offset,
                    ap=[list(bias_b.ap[0]), [0, A], list(bias_b.ap[1])],
                )
                # + bias on DVE (Pool tensor ops share SBUF ports with DVE and
                # halve its throughput while they run — measured, not worth it)
                nc.vector.tensor_add(out=xt, in0=xt, in1=bias_ba)
                mvt = stats.tile([P, A, 2], dt.float32, tag="mv")
                sig = stats.tile([P, A], dt.float32, tag="sig")
                for a in range(A):
                    st = stats.tile([P, 2, 6], dt.float32, tag="st")
                    xss = xt[:, a, :].rearrange("p (n f) -> p n f", f=512)
                    nc.vector.bn_stats(out=st[:, 0, :], in_=xss[:, 0, :])
                    nc.vector.bn_stats(out=st[:, 1, :], in_=xss[:, 1, :])
                    nc.vector.bn_aggr(out=mvt[:, a, :], in_=st)
                # rsigma = 1/sqrt(var + eps): sqrt half on ACT
                nc.scalar.activation(
                    out=sig,
                    in_=mvt[:, :, 1],
                    func=mybir.ActivationFunctionType.Sqrt,
                    bias=eps_t,
                    scale=1.0,
                )
                return (j, xt, mvt, sig)

            def stageC(s):
                j, xt, mvt, sig = s
                nc.vector.reciprocal(out=sig, in_=sig)
                nmr = stats.tile([P, A], dt.float32, tag="nmr")
                nc.vector.tensor_mul(out=nmr, in0=mvt[:, :, 0], in1=sig)
                nc.vector.tensor_scalar(
                    out=nmr,
                    in0=nmr,
                    scalar1=-1.0,
                    scalar2=None,
                    op0=mybir.AluOpType.mult,
                )
                lnt = work.tile([P, A, H], dt.float32, tag="ln")
                for a in range(A):
                    # t = (bda - mu) * rsigma = bda*rsigma + nmr, on ACT
                    nc.scalar.activation(
                        out=lnt[:, a, :],
                        in_=xt[:, a, :],
                        func=mybir.ActivationFunctionType.Identity,
                        bias=nmr[:, a : a + 1],
                        scale=sig[:, a : a + 1],
                    )
                return (j, xt, lnt)

            def stageD(s):
                j, xt, lnt = s
                # ln = t * gamma + beta (DVE)
                nc.vector.tensor_mul(out=lnt, in0=lnt, in1=gamma_ba)
                nc.vector.tensor_add(out=lnt, in0=lnt, in1=beta_ba)
                # per-partition |ln| max, one column per row-block
                nc.vector.tensor_reduce(
                    out=amax_acc[:, j * A : (j + 1) * A],
                    in_=lnt,
                    axis=mybir.AxisListType.X,
                    op=mybir.AluOpType.max,
                    apply_absolute_value=True,
                )
                # fp8 e4m3 cast on the scalar engine, whole supertile
                f8t = work.tile([P, A, H], dt.float8e4, tag="f8")
                nc.scalar.copy(out=f8t, in_=lnt)
                nc.sync.dma_start(out=bv[j], in_=xt)
                nc.sync.dma_start(out=fv[j], in_=f8t)

            # Emission order within an iteration is [A, D, C, B]: stage B's
            # bias-add lands ~7 us into the cycle so its SWDGE accum (issued
            # one iteration earlier, ~16 us latency) is already complete, and
            # stage C's ACT scale-shift lands mid-cycle so stage D's gamma
            # never waits on it next iteration.
            sa = sb = sc = None
            for j in range(J + 3):
                nsa = stageA(j) if j < J else None
                if sc is not None:
                    stageD(sc)
                nsc = stageC(sb) if sb is not None else None
                nsb = stageB(sa) if sa is not None else None
                sc = nsc
                sb = nsb
                sa = nsa

            amax_pp = stats.tile([P, 1], dt.float32, tag="apc")
            nc.vector.tensor_reduce(
                out=amax_pp,
                in_=amax_acc,
                axis=mybir.AxisListType.X,
                op=mybir.AluOpType.max,
            )
            nc.sync.dma_start(out=amax_out[:, :], in_=amax_pp)
    # Run the Bacc compile passes (register allocation, event-semaphore
    # legalization); run_bass_via_pjrt serializes nc.m as-is.
    nc.finalize()
    return nc


_NC_CACHE: dict = {}


def _get_nc() -> bass.Bass:
    if "nc" not in _NC_CACHE:
        _NC_CACHE["nc"] = build_nc()
    return _NC_CACHE["nc"]


def _run(in_maps, trace=False, **kwargs):
    from concourse.bass_utils import run_bass_kernel_spmd

    return run_bass_kernel_spmd(
        _get_nc(), in_maps, list(range(N_CORES)), trace=trace, **kwargs
    )


def _make_in_maps(x, bias, residual, ln_weight, ln_bias):
    x2 = np.ascontiguousarray(np.asarray(x, dtype=np.float32).reshape(R_FULL, H))
    r2 = np.ascontiguousarray(np.asarray(residual, dtype=np.float32).reshape(R_FULL, H))
    bias = np.ascontiguousarray(np.asarray(bias, dtype=np.float32))
    w = np.ascontiguousarray(np.asarray(ln_weight, dtype=np.float32))
    b = np.ascontiguousarray(np.asarray(ln_bias, dtype=np.float32))
    in_maps = []
    for i in range(N_CORES):
        rows = slice(i * R_CORE, (i + 1) * R_CORE)
        in_maps.append(
            {
                "x": x2[rows],
                "residual": r2[rows],
                "bias": bias,
                "ln_weight": w,
                "ln_bias": b,
            }
        )
    return in_maps


def _gather(results):
    bda = np.concatenate([r["bda_out"] for r in results], axis=0)
    fp8 = np.concatenate([r["fp8_out"] for r in results], axis=0)
    # TRN float8e4 matches OCP e4m3fn bit-for-bit over its finite range.
    fp8 = fp8.view(ml_dtypes.float8_e4m3fn)
    amax = np.float32(max(np.max(r["amax_out"]) for r in results))
    return bda, fp8, amax


def kernel(x, bias, residual, ln_weight, ln_bias):
    in_maps = _make_in_maps(x, bias, residual, ln_weight, ln_bias)
    out = _run(in_maps)
    return _gather(out.results)


# revision 52
# speedup vs baseline: 1.0477x; 1.0477x over previous
"""BiasAdd + LayerNorm + FP8 quantization kernel for Trainium2 (Bass/Tile).

Reference computation (see problem reference.py):
    bda  = residual + (x + bias)                 # [B,S,H] -> flattened [B*S, H]
    ln   = layernorm(bda) * ln_weight + ln_bias  # fp32
    amax = max(|ln|)
    fp8  = clip(ln, +-448).astype(float8_e4m3fn)
    returns (bda2, fp8, amax)

Sharding: data-parallel over the flattened token dim (32768 rows) across
8 NeuronCores -> 4096 rows/core. bias/ln_weight/ln_bias replicated.
amax: per-partition partial maxima [128] per core, final max on host.

Engine split (per 2-block supertile, chosen from measured costs):
  - x+residual: computed during the residual load (SWDGE accum DMA,
    4 software-DGE queues so the ~13 us RMW transfers overlap)
  - +bias, *gamma, +beta, |ln| max, mean/var (bn_stats): DVE, batched
    across the supertile (fp32 tensor_tensor is capped at 1 elem/cycle;
    per-column operands can only run on DVE -- Pool shares DVE's SBUF
    ports and the PE's fp32 passthrough is slower, both measured)
  - sqrt, (bda-mu)*rsigma scale-shift, fp8 cast: ACT
Four-stage software pipeline across supertiles hides the cross-engine
latency. The first 3 supertiles bypass the SWDGE accum (HWDGE loads + DVE
add) to shorten pipeline fill. Measured ~227-231 us/core vs the
~150 us HBM roofline (52 MiB
per core at ~358 GB/s per-NC).
"""

import sys

import numpy as np

_TRN_REPO = "/opt/trn_rl_repo"
if _TRN_REPO not in sys.path:
    sys.path.insert(0, _TRN_REPO)

import ml_dtypes  # noqa: E402
import concourse.bass as bass  # noqa: E402
import concourse.bacc as bacc  # noqa: E402
import concourse.tile as tile  # noqa: E402
from concourse import mybir  # noqa: E402

EPS = 1e-5
H = 1024
P = 128
N_CORES = 8
R_FULL = 8 * 4096  # B * S
R_CORE = R_FULL // N_CORES  # 4096 rows per core
BLOCKS_PER_TILE = 2  # A: 128-row blocks per supertile


def _bcast(ap, dims):
    """Prepend broadcast (step-0) dims to an AP."""
    return bass.AP(
        tensor=ap.tensor,
        offset=ap.offset,
        ap=[[0, n] for n in dims] + list(ap.ap),
    )


def build_nc(rows: int = R_CORE, blocks_per_tile: int = BLOCKS_PER_TILE) -> bass.Bass:
    """One NeuronCore's program: bias-add + layernorm + fp8 over [rows, H]."""
    dt = mybir.dt
    A = blocks_per_tile
    assert rows % (P * A) == 0
    nblk = rows // P
    J = nblk // A

    # Bacc (not plain Bass): its finalize() runs generate_event_semaphores(),
    # which splits multi-semaphore waits to satisfy the 1-wait-per-instruction
    # hardware constraint that Tile-emitted code otherwise violates.
    # 4 SWDGE queues: the residual accum-DMAs would otherwise serialize on
    # one software-DGE queue (~15 us per 1 MiB RMW transfer) and pace the
    # whole pipeline.
    nc = bacc.Bacc(num_swdge_queues=4)
    x = nc.declare_dram_parameter("x", [rows, H], dt.float32, isOutput=False)
    res = nc.declare_dram_parameter("residual", [rows, H], dt.float32, isOutput=False)
    bias = nc.declare_dram_parameter("bias", [H], dt.float32, isOutput=False)
    gamma = nc.declare_dram_parameter("ln_weight", [H], dt.float32, isOutput=False)
    beta = nc.declare_dram_parameter("ln_bias", [H], dt.float32, isOutput=False)
    bda_out = nc.declare_dram_parameter("bda_out", [rows, H], dt.float32, isOutput=True)
    fp8_out = nc.declare_dram_parameter("fp8_out", [rows, H], dt.float8e4, isOutput=True)
    amax_out = nc.declare_dram_parameter(
        "amax_out", [P, nblk], dt.float32, isOutput=True
    )

    # [rows, H] viewed as J supertiles of [P partitions, A row-blocks, H]
    xv = x[:].rearrange("(j a p) h -> j p a h", p=P, a=A)
    rv = res[:].rearrange("(j a p) h -> j p a h", p=P, a=A)
    bv = bda_out[:].rearrange("(j a p) h -> j p a h", p=P, a=A)
    fv = fp8_out[:].rearrange("(j a p) h -> j p a h", p=P, a=A)

    with tile.TileContext(nc) as tc:
        with (
            tc.tile_pool(name="consts", bufs=1) as consts,
            tc.tile_pool(name="io", bufs=6) as io,
            tc.tile_pool(name="work", bufs=5) as work,
            tc.tile_pool(name="stats", bufs=8) as stats,
            tc.tile_pool(name="rpool", bufs=3) as rpool,
        ):
            # Broadcast bias/gamma/beta across all 128 partitions once.
            bias_b = consts.tile([P, H], dt.float32, tag="bias_b")
            gamma_b = consts.tile([P, H], dt.float32, tag="gamma_b")
            beta_b = consts.tile([P, H], dt.float32, tag="beta_b")
            for tgt, src in ((bias_b, bias), (gamma_b, gamma), (beta_b, beta)):
                nc.sync.dma_start(out=tgt, in_=_bcast(src[:], [P]))
            eps_t = consts.tile([P, 1], dt.float32, tag="eps")
            nc.vector.memset(eps_t, EPS)
            # Per-partition running |ln| maxima, one column per row-block.
            amax_acc = consts.tile([P, nblk], dt.float32, tag="amax_acc")

            gamma_ba = bass.AP(
                tensor=gamma_b.tensor,
                offset=gamma_b.offset,
                ap=[list(gamma_b.ap[0]), [0, A], list(gamma_b.ap[1])],
            )
            beta_ba = bass.AP(
                tensor=beta_b.tensor,
                offset=beta_b.offset,
                ap=[list(beta_b.ap[0]), [0, A], list(beta_b.ap[1])],
            )
            # Four-stage software pipeline over supertiles so DVE never
            # stalls on a DMA or ACT round trip:
            #   stage A (j):   x load + residual SWDGE-accum issue (prefetch)
            #   stage B (j-1): +bias (DVE), bn_stats (DVE), sqrt (ACT)
            #   stage C (j-2): recip + nmr (DVE), scale-shift (ACT)
            #   stage D (j-3): gamma/beta/|max| (DVE), cast (ACT), stores
            def stageA(j):
                xt = io.tile([P, A, H], dt.float32, tag="xt")
                nc.sync.dma_start(out=xt, in_=xv[j])
                if j < 3:
                    # ramp peeling: during pipeline fill the SWDGE accum's
                    # ~16 us latency gates the first supertiles while DVE sits
                    # idle — load residual over HWDGE (~3 us) and add on DVE
                    rt = rpool.tile([P, A, H], dt.float32, tag="rt")
                    nc.sync.dma_start(out=rt, in_=rv[j])
                    return (j, xt, rt)
                # x+residual computed by a SWDGE accum DMA
                nc.gpsimd.dma_start(out=xt, in_=rv[j], accum_op=mybir.AluOpType.add)
                return (j, xt, None)

            def stageB(s):
                j, xt, rt = s
                if rt is not None:
                    nc.vector.tensor_add(out=xt, in0=xt, in1=rt)
                bias_ba = bass.AP(
                    tensor=bias_b.tensor,
                    offset=bias_b.offset,
                    ap=[list(bias_b.ap[0]), [0, A], list(bias_b.ap[1])],
                )
                # + bias on DVE (Pool tensor ops share SBUF ports with DVE and
                # halve its throughput while they run — measured, not worth it)
                nc.vector.tensor_add(out=xt, in0=xt, in1=bias_ba)
                mvt = stats.tile([P, A, 2], dt.float32, tag="mv")
                sig = stats.tile([P, A], dt.float32, tag="sig")
                for a in range(A):
                    st = stats.tile([P, 2, 6], dt.float32, tag="st")
                    xss = xt[:, a, :].rearrange("p (n f) -> p n f", f=512)
                    nc.vector.bn_stats(out=st[:, 0, :], in_=xss[:, 0, :])
                    nc.vector.bn_stats(out=st[:, 1, :], in_=xss[:, 1, :])
                    nc.vector.bn_aggr(out=mvt[:, a, :], in_=st)
                # rsigma = 1/sqrt(var + eps): sqrt half on ACT
                nc.scalar.activation(
                    out=sig,
                    in_=mvt[:, :, 1],
                    func=mybir.ActivationFunctionType.Sqrt,
                    bias=eps_t,
                    scale=1.0,
                )
                return (j, xt, mvt, sig)

            def stageC(s):
                j, xt, mvt, sig = s
                nc.vector.reciprocal(out=sig, in_=sig)
                nmr = stats.tile([P, A], dt.float32, tag="nmr")
                nc.vector.tensor_mul(out=nmr, in0=mvt[:, :, 0], in1=sig)
                nc.vector.tensor_scalar(
                    out=nmr,
                    in0=nmr,
                    scalar1=-1.0,
                    scalar2=None,
                    op0=mybir.AluOpType.mult,
                )
                lnt = work.tile([P, A, H], dt.float32, tag="ln")
                for a in range(A):
                    # t = (bda - mu) * rsigma = bda*rsigma + nmr, on ACT
                    nc.scalar.activation(
                        out=lnt[:, a, :],
                        in_=xt[:, a, :],
                        func=mybir.ActivationFunctionType.Identity,
                        bias=nmr[:, a : a + 1],
                        scale=sig[:, a : a + 1],
                    )
                return (j, xt, lnt)

            def stageD(s):
                j, xt, lnt = s
                # ln = t * gamma + beta (DVE)
                nc.vector.tensor_mul(out=lnt, in0=lnt, in1=gamma_ba)
                nc.vector.tensor_add(out=lnt, in0=lnt, in1=beta_ba)
                # per-partition |ln| max, one column per row-block
                nc.vector.tensor_reduce(
                    out=amax_acc[:, j * A : (j + 1) * A],
                    in_=lnt,
                    axis=mybir.AxisListType.X,
                    op=mybir.AluOpType.max,
                    apply_absolute_value=True,
                )
                # fp8 e4m3 cast on the scalar engine, whole supertile
                f8t = work.tile([P, A, H], dt.float8e4, tag="f8")
                nc.scalar.copy(out=f8t, in_=lnt)
                nc.sync.dma_start(out=bv[j], in_=xt)
                nc.sync.dma_start(out=fv[j], in_=f8t)

            # Emission order within an iteration is [A, D, C, B]: stage B's
            # bias-add lands ~7 us into the cycle so its SWDGE accum (issued
            # one iteration earlier, ~16 us latency) is already complete, and
            # stage C's ACT scale-shift lands mid-cycle so stage D's gamma
            # never waits on it next iteration.
            sa = sb = sc = None
            for j in range(J + 3):
                nsa = stageA(j) if j < J else None
                if sc is not None:
                    stageD(sc)
                nsc = stageC(sb) if sb is not None else None
                nsb = stageB(sa) if sa is not None else None
                sc = nsc
                sb = nsb
                sa = nsa

            # ship the whole per-block accumulator; host takes the final max
            nc.sync.dma_start(out=amax_out[:, :], in_=amax_acc)
    # Run the Bacc compile passes (register allocation, event-semaphore
    # legalization); run_bass_via_pjrt serializes nc.m as-is.
    nc.finalize()
    return nc


_NC_CACHE: dict = {}


def _get_nc() -> bass.Bass:
    if "nc" not in _NC_CACHE:
        _NC_CACHE["nc"] = build_nc()
    return _NC_CACHE["nc"]


def _run(in_maps, trace=False, **kwargs):
    from concourse.bass_utils import run_bass_kernel_spmd

    return run_bass_kernel_spmd(
        _get_nc(), in_maps, list(range(N_CORES)), trace=trace, **kwargs
    )


def _make_in_maps(x, bias, residual, ln_weight, ln_bias):
    x2 = np.ascontiguousarray(np.asarray(x, dtype=np.float32).reshape(R_FULL, H))
    r2 = np.ascontiguousarray(np.asarray(residual, dtype=np.float32).reshape(R_FULL, H))
    bias = np.ascontiguousarray(np.asarray(bias, dtype=np.float32))
    w = np.ascontiguousarray(np.asarray(ln_weight, dtype=np.float32))
    b = np.ascontiguousarray(np.asarray(ln_bias, dtype=np.float32))
    in_maps = []
    for i in range(N_CORES):
        rows = slice(i * R_CORE, (i + 1) * R_CORE)
        in_maps.append(
            {
                "x": x2[rows],
                "residual": r2[rows],
                "bias": bias,
                "ln_weight": w,
                "ln_bias": b,
            }
        )
    return in_maps


def _gather(results):
    bda = np.concatenate([r["bda_out"] for r in results], axis=0)
    fp8 = np.concatenate([r["fp8_out"] for r in results], axis=0)
    # TRN float8e4 matches OCP e4m3fn bit-for-bit over its finite range.
    fp8 = fp8.view(ml_dtypes.float8_e4m3fn)
    amax = np.float32(max(np.max(r["amax_out"]) for r in results))
    return bda, fp8, amax


def kernel(x, bias, residual, ln_weight, ln_bias):
    in_maps = _make_in_maps(x, bias, residual, ln_weight, ln_bias)
    out = _run(in_maps)
    return _gather(out.results)
